# revision 1
# baseline (speedup 1.0000x reference)
"""Trainium2 Bass kernel for MeanGaussianExactFlow.

Math notes (derived from the nn.Module reference):
  - z_corrected == z exactly (the x_mean @ H.T terms cancel), so x_mean is
    never needed.
  - inv(lam*H@P@H.T + sigma_b^2 I) for all batches shares one fixed symmetric
    matrix S = lam*H@V@H.T.  With S = Q diag(e) Q^T (one tiny host-side 64x64
    eigendecomposition), the batched inverse is Q diag(1/(e+sigma_b^2)) Q^T.
  - A_b = U G_b W with U = -0.5*V@H.T@Q [D,M], W = Q.T@H [M,D],
    G_b = diag(1/(e_m + sigma_b^2)).
  - f_b = x_b @ A_b^T + b_b^T, with b_b computed from z_b, sigma_b via a few
    [<=128, BLOC] matmuls on device.

Device work per core (32 batches, pure data parallel over B):
  per batch: 8 PE transposes (x tiles) -> PSUM -> ACT copy -> 8 PE matmuls
  vs A_b^T -> DVE bias-add PSUM->SBUF -> f store. x loads are 1 MB (2
  batches) on the SP HWDGE ring; f stores are per-batch on SWDGE.
"""

import numpy as np

B, N, D, M = 256, 1024, 128, 64
NCORES = 8
BLOC = B // NCORES  # 32 batches per core
NT = N // 128  # 8 n-tiles per batch
GW = 4  # tiles per psum group ([128, 512] = one bank)
NG = NT // GW  # 2 groups per batch
BB = 2  # batches per DMA
AT_HOIST = 32  # A^T preps emitted before the main loop
AT_LOOKAHEAD = 6  # remaining A^T preps emitted this many batches early

# packed const layout (columns in a [128, CW] fp32 tensor)
_C_WT = 0          # W^T            [128, 64]   cols 0:64
_C_WRAW = 64       # W              [64, 128]   cols 64:192
_C_UT = 192        # U^T            [64, 128]   cols 192:320
_C_PHTT = 320      # (V H^T)^T      [64, 128]   cols 320:448
_C_EIG = 448       # eigenvalues    [1, 64]     cols 448:512
_C_WMU = 512       # W @ mu         [64, 1]     col  512
_C_ZT = 513        # z^T            [64, 32]    cols 513:545
_C_SIG = 545       # sigma          [1, 32]     cols 545:577
CW = 577


_PROGRAM_CACHE = {}


def _build_program(lam: float):
    if lam in _PROGRAM_CACHE:
        return _PROGRAM_CACHE[lam]
    import concourse.mybir as mybir
    import concourse.tile as tile
    from concourse import bacc
    from concourse.masks import make_identity
    from contextlib import ExitStack

    fp32 = mybir.dt.float32
    nc = bacc.Bacc("TRN2", target_bir_lowering=False, debug=False)

    x_d = nc.dram_tensor("x", [BLOC, N, D], fp32, kind="ExternalInput")
    c_d = nc.dram_tensor("consts", [128, CW], fp32, kind="ExternalInput")
    f_d = nc.dram_tensor("f", [BLOC, N, D], fp32, kind="ExternalOutput")
    bsc_d = nc.dram_tensor("bscratch", [BLOC, D], fp32)

    with tile.TileContext(nc) as tc, ExitStack() as ctx:
        const = ctx.enter_context(tc.tile_pool(name="const", bufs=1))
        prep_sb = ctx.enter_context(tc.tile_pool(name="prep_sb", bufs=1))

        ident = const.tile([128, 128], fp32)
        make_identity(nc, ident)
        ones_row = const.tile([1, 128], fp32)
        nc.any.memset(ones_row[:], 1.0)

        call = const.tile([128, CW], fp32)
        # load the G-chain inputs (eig, z^T, sigma) first: they gate the
        # prep chain and all A^T matmuls
        nc.sync.dma_start(call[:M, _C_EIG:], c_d.ap()[:M, _C_EIG:])
        nc.sync.dma_start(call[:, :_C_EIG], c_d.ap()[:, :_C_EIG])
        wt_s = call[:, _C_WT : _C_WT + M]            # [128, 64]
        wraw_s = call[:M, _C_WRAW : _C_WRAW + D]     # [64, 128]
        ut_s = call[:M, _C_UT : _C_UT + D]           # [64, 128]
        phtt_s = call[:M, _C_PHTT : _C_PHTT + D]     # [64, 128]
        eig_s = call[:1, _C_EIG : _C_EIG + M]        # [1, 64]
        wmu_s = call[:M, _C_WMU : _C_WMU + 1]        # [64, 1]
        zt_s = call[:M, _C_ZT : _C_ZT + BLOC]        # [64, 32]
        sig_s = call[:1, _C_SIG : _C_SIG + BLOC]     # [1, 32]

        # ---- prep chain: G matrix + bias vectors, col-layout [*, BLOC] ----
        gmat = prep_sb.tile([M, BLOC], fp32)
        ball = prep_sb.tile([D, BLOC], fp32)
        with tc.tile_pool(name="prep_ps", bufs=1, space="PSUM") as prep_ps:
            sig2 = prep_sb.tile([1, BLOC], fp32)
            nc.vector.tensor_mul(sig2[:], sig_s[:], sig_s[:])
            isig2 = prep_sb.tile([1, BLOC], fp32)
            nc.vector.reciprocal(isig2[:], sig2[:])

            # Gden[m, b] = eig_m + sig2_b (two rank-1 matmuls into one psum)
            gden_ps = prep_ps.tile([M, BLOC], fp32, tag="pp64")
            nc.tensor.matmul(
                gden_ps[:], eig_s[:], ones_row[:, :BLOC], start=True, stop=False
            )
            nc.tensor.matmul(
                gden_ps[:], ones_row[:, :M], sig2[:], start=False, stop=True
            )
            nc.vector.reciprocal(gmat[:], gden_ps[:])

            # SigM[d, b] = 1/sig2_b broadcast down 128 partitions (rank-1)
            sigm_ps = prep_ps.tile([D, BLOC], fp32, tag="pp128")
            nc.tensor.matmul(sigm_ps[:], ones_row[:], isig2[:], start=True, stop=True)
            sigm = prep_sb.tile([D, BLOC], fp32)
            nc.scalar.copy(sigm[:], sigm_ps[:])

            # t1 = PHT @ z / sig2   [D, BLOC]
            t1_ps = prep_ps.tile([D, BLOC], fp32, tag="pp128")
            nc.tensor.matmul(t1_ps[:], phtt_s, zt_s, start=True, stop=True)
            t1s = prep_sb.tile([D, BLOC], fp32)
            nc.vector.tensor_mul(t1s[:], t1_ps[:], sigm[:])

            # r1 = W @ t1  [M, BLOC]
            r1_ps = prep_ps.tile([M, BLOC], fp32, tag="pp64")
            nc.tensor.matmul(r1_ps[:], wt_s, t1s[:], start=True, stop=True)
            r1gl = prep_sb.tile([M, BLOC], fp32)
            nc.vector.scalar_tensor_tensor(
                r1gl[:], r1_ps[:], float(lam), gmat[:],
                mybir.AluOpType.mult, mybir.AluOpType.mult,
            )
            gwmu = prep_sb.tile([M, BLOC], fp32)
            nc.vector.tensor_scalar_mul(gwmu[:], gmat[:], wmu_s)
            rhs5 = prep_sb.tile([M, BLOC], fp32)
            nc.vector.tensor_add(rhs5[:], r1gl[:], gwmu[:])
            # q = U @ rhs5; s = t1s + q
            q_ps = prep_ps.tile([D, BLOC], fp32, tag="pp128")
            nc.tensor.matmul(q_ps[:], ut_s, rhs5[:], start=True, stop=True)
            s_sb = prep_sb.tile([D, BLOC], fp32)
            nc.vector.tensor_add(s_sb[:], q_ps[:], t1s[:])
            # r2 = W @ s; r2g = (r2*2lam).*G; q2 = U @ r2g; Ball = s + q2
            r2_ps = prep_ps.tile([M, BLOC], fp32, tag="pp64")
            nc.tensor.matmul(r2_ps[:], wt_s, s_sb[:], start=True, stop=True)
            r2g = prep_sb.tile([M, BLOC], fp32)
            nc.vector.scalar_tensor_tensor(
                r2g[:], r2_ps[:], float(2.0 * lam), gmat[:],
                mybir.AluOpType.mult, mybir.AluOpType.mult,
            )
            q2_ps = prep_ps.tile([D, BLOC], fp32, tag="pp128")
            nc.tensor.matmul(q2_ps[:], ut_s, r2g[:], start=True, stop=True)
            nc.vector.tensor_add(ball[:], q2_ps[:], s_sb[:])

            # bias rows: transpose once, bounce via DRAM to a flat row on
            # partition 0 so per-batch rows are partition_broadcast-able
            ballt_ps = prep_ps.tile([BLOC, D], fp32, tag="pp128")
            nc.tensor.transpose(ballt_ps[:], ball[:], ident[:])
            ballt_sb = prep_sb.tile([BLOC, D], fp32)
            nc.scalar.copy(ballt_sb[:], ballt_ps[:])
            nc.sync.dma_start(bsc_d.ap(), ballt_sb[:])
            ballf = prep_sb.tile([1, BLOC * D], fp32)
            nc.sync.dma_start(ballf[:], bsc_d.ap().rearrange("a b -> (a b)").unsqueeze(0))

        # ---- main loop pools (prep PSUM released; 8 banks available) ----
        xb_pool = ctx.enter_context(tc.tile_pool(name="xb", bufs=4))
        fb_pool = ctx.enter_context(tc.tile_pool(name="fb", bufs=6))
        xts_pool = ctx.enter_context(tc.tile_pool(name="xts", bufs=4))
        at_pool = ctx.enter_context(tc.tile_pool(name="ats", bufs=BLOC))
        wg_pool = ctx.enter_context(tc.tile_pool(name="wg", bufs=4))
        bb_pool = ctx.enter_context(tc.tile_pool(name="bb", bufs=4))
        xt_ps_pool = ctx.enter_context(tc.tile_pool(name="xtps", bufs=3, space="PSUM"))
        f_ps_pool = ctx.enter_context(tc.tile_pool(name="fps", bufs=3, space="PSUM"))
        misc_ps_pool = ctx.enter_context(
            tc.tile_pool(name="miscps", bufs=2, space="PSUM")
        )

        # A_b^T prep, hoisted ahead of each batch's compute (see emit order
        # below): fills PE idle while DMA streams x, shortens PE period
        at_sbs = {}

        def emit_at(b):
            wg = wg_pool.tile([M, D], fp32)
            nc.scalar.mul(wg[:], wraw_s, gmat[:, b : b + 1])
            at_ps = misc_ps_pool.tile([D, D], fp32, tag="mps")
            nc.tensor.matmul(at_ps[:], wg[:], ut_s, start=True, stop=True)
            at_sb = at_pool.tile([D, D], fp32, tag="at")
            nc.vector.tensor_copy(at_sb[:], at_ps[:])
            at_sbs[b] = at_sb

        emit_at(0)
        emit_at(1)

        for bp in range(0, BLOC, BB):
            xb = xb_pool.tile([128, BB, NT, D], fp32, tag="xb")
            if bp == 0:
                for bi in range(BB):
                    nc.sync.dma_start(
                        xb[:, bi, :, :],
                        x_d.ap()[bp + bi].rearrange("(p t) d -> p t d", p=128),
                    )
            else:
                nc.sync.dma_start(
                    xb[:],
                    x_d.ap()[bp : bp + BB].rearrange("c (p t) d -> p c t d", p=128),
                )
            if bp == 0:
                # enough A_b^T preps to fill the startup PE idle; the rest
                # are emitted staggered (lookahead) inside the batch loop
                for b2 in range(2, AT_HOIST):
                    emit_at(b2)
            for bi in range(BB):
                b = bp + bi
                fb = fb_pool.tile([128, NT, D], fp32)
                if b + AT_LOOKAHEAD < BLOC and (b + AT_LOOKAHEAD) not in at_sbs:
                    emit_at(b + AT_LOOKAHEAD)
                if b not in at_sbs:
                    emit_at(b)
                at_sb = at_sbs[b]

                # bias row at partition 0 -> Pool-engine broadcast
                bb_sb = bb_pool.tile([128, D], fp32)
                nc.gpsimd.partition_broadcast(
                    bb_sb[:], ballf[:, b * D : (b + 1) * D]
                )

                gw = 2 if b == 0 else GW  # finer first batch: shorter fill
                for g in range(NT // gw):
                    xt_ps = xt_ps_pool.tile([128, GW, 128], fp32)
                    for j in range(gw):
                        t = g * gw + j
                        nc.tensor.transpose(xt_ps[:, j, :], xb[:, bi, t, :], ident[:])
                    xt_sb = xts_pool.tile([128, GW, 128], fp32)
                    nc.scalar.copy(xt_sb[:, :gw, :], xt_ps[:, :gw, :])
                    f_ps = f_ps_pool.tile([128, GW, D], fp32)
                    for j in range(gw):
                        nc.tensor.matmul(
                            f_ps[:, j, :], xt_sb[:, j, :], at_sb[:],
                            start=True, stop=True,
                        )
                    nc.vector.tensor_add(
                        fb[:, g * gw : (g + 1) * gw, :],
                        f_ps[:, :gw, :],
                        bb_sb[:, None, :].broadcast_to([128, gw, D]),
                    )

                if b >= BLOC - 2:
                    # tail: HWDGE per-group stores to shorten the critical path
                    for g in range(NG):
                        nc.sync.dma_start(
                            f_d.ap()[b].rearrange("(p t) d -> p t d", p=128)[
                                :, g * GW : (g + 1) * GW, :
                            ],
                            fb[:, g * GW : (g + 1) * GW, :],
                        )
                else:
                    feng = nc.gpsimd if b % 2 == 0 else nc.scalar
                    feng.dma_start(
                        f_d.ap()[b].rearrange("(p t) d -> p t d", p=128), fb[:]
                    )

    nc.compile()
    _PROGRAM_CACHE[lam] = nc
    return nc


def kernel(lam, x, H, sigma, z, V_prior, mu_prior):
    lam = float(np.asarray(lam))
    x = np.ascontiguousarray(np.asarray(x, dtype=np.float32))
    H = np.asarray(H, dtype=np.float32)
    sigma = np.asarray(sigma, dtype=np.float32)
    z = np.asarray(z, dtype=np.float32)
    V_prior = np.asarray(V_prior, dtype=np.float32)
    mu_prior = np.asarray(mu_prior, dtype=np.float32)

    # Tiny shared prep in float64 (one 64x64 eigendecomposition)
    H64 = H.astype(np.float64)
    V64 = V_prior.astype(np.float64)
    PHT = V64 @ H64.T                      # [D, M]
    S = lam * (H64 @ PHT)                  # [M, M] symmetric PSD
    S = 0.5 * (S + S.T)
    e, Q = np.linalg.eigh(S)
    U_hat = -0.5 * (PHT @ Q)               # [D, M]
    W = Q.T @ H64                          # [M, D]
    Wmu = W @ mu_prior.astype(np.float64)  # [M]

    f32 = np.float32
    base = np.zeros((128, CW), dtype=f32)
    base[:, _C_WT : _C_WT + M] = W.T.astype(f32)
    base[:M, _C_WRAW : _C_WRAW + D] = W.astype(f32)
    base[:M, _C_UT : _C_UT + D] = U_hat.T.astype(f32)
    base[:M, _C_PHTT : _C_PHTT + D] = PHT.T.astype(f32)
    base[:1, _C_EIG : _C_EIG + M] = e.astype(f32)[None, :]
    base[:M, _C_WMU : _C_WMU + 1] = Wmu.astype(f32)[:, None]

    nc = _build_program(lam)

    in_maps = []
    for c in range(NCORES):
        lo, hi = c * BLOC, (c + 1) * BLOC
        cc = base.copy()
        cc[:M, _C_ZT : _C_ZT + BLOC] = z[lo:hi].T.astype(f32)
        cc[:1, _C_SIG : _C_SIG + BLOC] = sigma[lo:hi][None, :].astype(f32)
        in_maps.append(dict(x=np.ascontiguousarray(x[lo:hi]), consts=cc))

    from concourse.bass_utils import run_bass_kernel_spmd

    res = run_bass_kernel_spmd(nc, in_maps, core_ids=list(range(NCORES)))
    out = np.concatenate([np.asarray(r["f"]) for r in res.results], axis=0)
    return out.astype(np.float32)



# revision 2
# speedup vs baseline: 1.9013x; 1.9013x over previous
"""Trainium2 Bass kernel for MeanGaussianExactFlow.

Math notes (derived from the nn.Module reference):
  - z_corrected == z exactly (the x_mean @ H.T terms cancel), so x_mean is
    never needed.
  - A_b = -0.5 * V H^T (lam H V H^T + sigma_b^2 I)^-1 H  and the bias chain
    b_b are tiny ([D,D] / [D] per batch) -> computed host-side in float64.
  - The only large compute is f_b = x_b @ A_b^T + b_b^T  (8.6 GFLOP total),
    which runs on device as f_b^T = A_b @ x_b^T + b_b, in bf16 with fp32
    PSUM accumulation (rel err ~4e-3, well under the 2e-2 gate).

Device work per core (32 batches, pure data parallel over B):
  per batch: 2 PE matmuls (lhsT = A_b^T stationary bf16 [128,128],
  rhs = x_b^T bf16 [128,512] moving) -> PSUM fp32 -> one fused
  bias-add + bf16-cast copy (alternating DVE/ACT) -> bf16 store.
  x^T arrives pre-transposed/pre-cast from host (bf16 halves HBM traffic
  in both directions vs the fp32 baseline; PE transposes are gone).
"""

import numpy as np

B, N, D, M = 256, 1024, 128, 64
NCORES = 8
BLOC = B // NCORES  # 32 batches per core
BB = 2  # batches per x-load DMA

_PROGRAM_CACHE = {}


def _build_program():
    if _PROGRAM_CACHE:
        return _PROGRAM_CACHE["nc"]
    import concourse.mybir as mybir
    import concourse.tile as tile
    from concourse import bacc
    from contextlib import ExitStack

    fp32 = mybir.dt.float32
    bf16 = mybir.dt.bfloat16
    nc = bacc.Bacc("TRN2", target_bir_lowering=False, debug=False)

    xt_d = nc.dram_tensor("xt", [BLOC, D, N], bf16, kind="ExternalInput")
    at_d = nc.dram_tensor("at", [D, BLOC * D], bf16, kind="ExternalInput")
    bias_d = nc.dram_tensor("bias", [D, BLOC], fp32, kind="ExternalInput")
    f_d = nc.dram_tensor("f", [BLOC, D, N], bf16, kind="ExternalOutput")

    with tile.TileContext(nc) as tc, ExitStack() as ctx:
        const = ctx.enter_context(tc.tile_pool(name="const", bufs=1))
        bias_s = const.tile([D, BLOC], fp32)
        nc.sync.dma_start(bias_s[:], bias_d.ap())
        at_s = const.tile([D, BLOC * D], bf16)
        nc.sync.dma_start(at_s[:], at_d.ap())

        xb_pool = ctx.enter_context(tc.tile_pool(name="xb", bufs=6))
        fb_pool = ctx.enter_context(tc.tile_pool(name="fb", bufs=8))
        ps_pool = ctx.enter_context(tc.tile_pool(name="ps", bufs=4, space="PSUM"))

        for bp in range(0, BLOC, BB):
            xb = xb_pool.tile([D, BB, N], bf16, tag="xb")
            nc.sync.dma_start(
                xb[:], xt_d.ap()[bp : bp + BB].rearrange("c e n -> e c n")
            )
            for bi in range(BB):
                b = bp + bi
                at_b = at_s[:, b * D : (b + 1) * D]
                ps = ps_pool.tile([D, 2, N // 2], fp32, tag="ps")
                for j in range(2):
                    nc.tensor.matmul(
                        ps[:, j, :],
                        at_b,
                        xb[:, bi, j * (N // 2) : (j + 1) * (N // 2)],
                        start=True,
                        stop=True,
                    )
                fb = fb_pool.tile([D, N], bf16, tag="fb")
                bcol = bias_s[:, b : b + 1]
                if b % 2 == 0:
                    nc.vector.tensor_scalar_add(
                        fb[:], ps[:].rearrange("e a n -> e (a n)"), bcol
                    )
                    nc.gpsimd.dma_start(f_d.ap()[b], fb[:])
                else:
                    nc.scalar.add(fb[:], ps[:].rearrange("e a n -> e (a n)"), bcol)
                    nc.scalar.dma_start(f_d.ap()[b], fb[:])

    nc.compile()
    _PROGRAM_CACHE["nc"] = nc
    return nc


def kernel(lam, x, H, sigma, z, V_prior, mu_prior):
    import jax
    import jax.numpy as jnp
    import ml_dtypes

    lam = float(np.asarray(lam))
    x = np.asarray(x, dtype=np.float32)
    H64 = np.asarray(H, dtype=np.float64)
    sigma64 = np.asarray(sigma, dtype=np.float64)
    z64 = np.asarray(z, dtype=np.float64)
    V64 = np.asarray(V_prior, dtype=np.float64)
    mu64 = np.asarray(mu_prior, dtype=np.float64)

    # ---- tiny per-batch prep in float64 (exact reference algebra) ----
    I_D = np.eye(D)
    I_M = np.eye(M)
    PHT = V64 @ H64.T                       # [D,M]
    HPHT = H64 @ PHT                        # [M,M]
    sig2 = sigma64**2
    Ainv = np.linalg.inv(lam * HPHT[None] + sig2[:, None, None] * I_M)  # [B,M,M]
    A = -0.5 * np.einsum("dm,bmn,ne->bde", PHT, Ainv, H64)              # [B,D,D]
    t1 = (PHT[None] / sig2[:, None, None]) @ z64[:, :, None]            # [B,D,1]
    tb1 = (I_D[None] + lam * A) @ t1
    tb2 = A @ mu64[None, :, None]
    bvec = (I_D[None] + 2.0 * lam * A) @ (tb1 + tb2)                    # [B,D,1]

    bf = ml_dtypes.bfloat16
    # A^T per batch, packed [D, BLOC*D]: at[e, b*D+d] = A_b[d, e]
    AT = np.ascontiguousarray(A.astype(np.float32).astype(bf))          # [B,D,D]
    bias = bvec[:, :, 0].astype(np.float32)                             # [B,D]

    # x^T per batch (one multithreaded jax-cpu pass for transpose+cast)
    cpu = jax.local_devices(backend="cpu")[0]
    with jax.default_device(cpu):
        to_bf = jax.jit(lambda a: jnp.transpose(a, (0, 2, 1)).astype(jnp.bfloat16))
        xt_all = np.asarray(to_bf(x))                                   # [B,D,N] bf16

    nc = _build_program()

    in_maps = []
    for c in range(NCORES):
        lo, hi = c * BLOC, (c + 1) * BLOC
        # at[e, b*D+d] = A_b[d, e] for local b
        at_pack = np.ascontiguousarray(
            AT[lo:hi].transpose(2, 0, 1).reshape(D, BLOC * D)
        )
        bias_pack = np.ascontiguousarray(bias[lo:hi].T)                 # [D,BLOC]
        in_maps.append(
            dict(
                xt=np.ascontiguousarray(xt_all[lo:hi]),
                at=at_pack,
                bias=bias_pack,
            )
        )

    from concourse.bass_utils import run_bass_kernel_spmd

    res = run_bass_kernel_spmd(nc, in_maps, core_ids=list(range(NCORES)))
    ft = np.stack([np.asarray(r["f"]) for r in res.results])            # [8,BLOC,D,N]
    with jax.default_device(cpu):
        back = jax.jit(
            lambda a: jnp.transpose(a.reshape(B, D, N), (0, 2, 1)).astype(jnp.float32)
        )
        out = np.asarray(back(ft))
    return out
